# revision 3
# baseline (speedup 1.0000x reference)
"""Trainium2 Bass kernel for nn_CrossAttention (B=4, Sx=4096, Sy=512, D=1024, H=8).

Sharding: 8 cores = (batch, query-half). Each core handles 2048 query rows of one
batch; K/V projections for that batch are computed locally (replicated across the
2 cores sharing a batch). The output projection is fully local, so no collectives
are needed; each core writes its own [2048, 1024] output slice.

All matmuls run in float32r (fp32 storage, reduced-precision PE multiply at
~bf16 rate). Layouts are arranged so no on-device transposes are needed:
  qT[d, q]   = Wq.T @ xT        (xT pre-transposed on host)
  kT[d, s]   = (Wk/sqrt(dh)).T @ yT
  v[s, d]    = yT.T @ Wv
  scT[s, q]  = kT_h_chunk.T @ qT_h            (per head, Sy chunks of 128)
  eT         = exp(scT)                        (no max-subtract: |scores| ~ O(1))
  Z[1, q]    = ones.T @ eT                     (softmax denom via PE reduction)
  aT[d, q]   = v_chunk.T @ eT, then aT *= (1/Z) broadcast via K=1 ones-matmul
  out[q, n]  = sum_h aT_h_chunk.T @ Wo_h + (bv @ Wo + bo)
"""
import sys
import types
import math
import numpy as np

sys.path.insert(0, "/opt/trn_rl_repo")

B, SX, SY, DE, DC, H, DH = 4, 4096, 512, 1024, 768, 8, 128
NCORES = 8
ROWS = B * SX // NCORES      # 2048 query rows per core
NW = ROWS // 512             # 4 windows of 512 rows
KQ = DE // 128               # 8 k-chunks for q/out projections
KY = DC // 128               # 6 k-chunks for k/v projections


def _install_shims():
    """antenv.axon_hooks is missing in this image; register the NTFF profile hook
    so trace=True works, and neuter the fish-bucket artifact upload."""
    if "antenv.axon_hooks" in sys.modules:
        return
    import antenv
    mod = types.ModuleType("antenv.axon_hooks")
    _h = [None]
    mod.set_axon_ntff_profile_hook = lambda h: _h.__setitem__(0, h)
    mod.get_axon_ntff_profile_hook = lambda: _h[0]
    sys.modules["antenv.axon_hooks"] = mod
    antenv.axon_hooks = mod
    try:
        from trn_agent_boot.trn_boot import _ntff_profile_via_ctypes
        mod.set_axon_ntff_profile_hook(
            _ntff_profile_via_ctypes("/opt/axon/libaxon_pjrt.so"))
    except Exception:
        pass
    from concourse import bass_utils
    bass_utils.upload_artifacts = lambda tmpdir: "local://" + tmpdir


_NC_CACHE = {}


def _build_nc():
    from concourse import bacc, mybir
    from concourse.tile import TileContext

    F32 = mybir.dt.float32
    F32R = mybir.dt.float32r
    Identity = mybir.ActivationFunctionType.Identity
    Copy = mybir.ActivationFunctionType.Copy
    Exp = mybir.ActivationFunctionType.Exp

    nc = bacc.Bacc(None, target_bir_lowering=False)
    xT = nc.dram_tensor("xT", [DE, ROWS], F32R, kind="ExternalInput")
    yT = nc.dram_tensor("yT", [DC, SY], F32R, kind="ExternalInput")
    wq = nc.dram_tensor("wq", [DE, DE], F32R, kind="ExternalInput")
    wk = nc.dram_tensor("wk", [DC, DE], F32R, kind="ExternalInput")
    wv = nc.dram_tensor("wv", [DC, DE], F32R, kind="ExternalInput")
    wo = nc.dram_tensor("wo", [DE, DE], F32R, kind="ExternalInput")
    bq_d = nc.dram_tensor("bq", [DE], F32, kind="ExternalInput")
    bk_d = nc.dram_tensor("bk", [DE], F32, kind="ExternalInput")
    bo_d = nc.dram_tensor("bo", [DE], F32, kind="ExternalInput")
    ones_d = nc.dram_tensor("ones", [128], F32R, kind="ExternalInput")
    out = nc.dram_tensor("out", [ROWS, DE], F32, kind="ExternalOutput")

    def chunked(ap, p=128):
        # DRAM [K, N] -> [p, K/p, N] view with the 128-partition dim first
        return ap.rearrange("(c p) n -> c p n", p=p).transpose([1, 0, 2])

    with TileContext(nc) as tc:
        with (
            tc.tile_pool(name="consts", bufs=1) as consts,
            tc.tile_pool(name="xwp", bufs=1) as xwp,
            tc.tile_pool(name="qtp", bufs=1) as qtp,
            tc.tile_pool(name="exp_p", bufs=2) as exp_p,
            tc.tile_pool(name="atp", bufs=1) as atp,
            tc.tile_pool(name="fop", bufs=2) as fop,
            tc.tile_pool(name="csp", bufs=1) as csp,
            tc.tile_pool(name="ps_a", bufs=2, space="PSUM") as ps_a,
            tc.tile_pool(name="ps_sc", bufs=2, space="PSUM") as ps_sc,
            tc.tile_pool(name="ps_at", bufs=2, space="PSUM") as ps_at,
            tc.tile_pool(name="ps_cs", bufs=1, space="PSUM") as ps_cs,
            tc.tile_pool(name="ps_bc", bufs=1, space="PSUM") as ps_bc,
        ):
            # ---- resident constants ----
            wq_t = consts.tile([128, KQ, DE], F32R)
            wo_t = consts.tile([128, KQ, DE], F32R)
            kt = consts.tile([128, H, SY], F32R)      # kT: [d-part, head, Sy]
            vt = consts.tile([128, SY // 128, DE], F32R)  # v: [Sy-part, Sy-chunk, d]
            bo_bc = consts.tile([128, DE], F32)
            bq_t = consts.tile([128, KQ], F32)
            bk_t = consts.tile([128, KQ], F32)
            ones_col = consts.tile([128, 1], F32R)
            ones_row = consts.tile([1, 128], F32R)

            nc.sync.dma_start(out=wq_t[:], in_=chunked(wq[:]))
            nc.sync.dma_start(out=wo_t[:], in_=chunked(wo[:]))
            nc.sync.dma_start(out=bo_bc[:], in_=bo_d[:].partition_broadcast(128))
            nc.sync.dma_start(out=bq_t[:], in_=bq_d[:].rearrange("(m p) -> p m", p=128))
            nc.sync.dma_start(out=bk_t[:], in_=bk_d[:].rearrange("(m p) -> p m", p=128))
            nc.sync.dma_start(out=ones_col[:], in_=ones_d[:].unsqueeze(1))
            nc.sync.dma_start(out=ones_row[:], in_=ones_d[:].unsqueeze(0))

            with tc.tile_pool(name="prologue", bufs=1) as pro:
                yt = pro.tile([128, KY, SY], F32R, tag="yt")
                nc.sync.dma_start(out=yt[:], in_=chunked(yT[:]))
                # stage Wk then Wv in 512-column halves through one shared slot
                for half in range(2):
                    wst = pro.tile([128, KY, 512], F32R, tag="wstage")
                    nc.sync.dma_start(out=wst[:], in_=chunked(wk[:, half * 512:(half + 1) * 512]))
                    # kT[d, s] = (Wk').T @ yT + bk'   (m-tiles covered by this half)
                    for mh in range(4):
                        m = half * 4 + mh
                        ps = ps_sc.tile([128, SY], F32, tag="sc")
                        for k in range(KY):
                            nc.tensor.matmul(ps[:], wst[:, k, mh * 128:(mh + 1) * 128],
                                             yt[:, k, :], start=(k == 0), stop=(k == KY - 1))
                        nc.scalar.activation(out=kt[:, m, :], in_=ps[:], func=Identity,
                                             bias=bk_t[:, m:m + 1], scale=1.0)
                for nh in range(2):
                    wst = pro.tile([128, KY, 512], F32R, tag="wstage")
                    nc.sync.dma_start(out=wst[:], in_=chunked(wv[:, nh * 512:(nh + 1) * 512]))
                    # v[s, d] = yT.T @ Wv  (bv folded into bo_eff on host)
                    for sy in range(SY // 128):
                        ps = ps_at.tile([128, 512], F32, tag="at")
                        for k in range(KY):
                            nc.tensor.matmul(ps[:], yt[:, k, sy * 128:(sy + 1) * 128],
                                             wst[:, k, :], start=(k == 0), stop=(k == KY - 1))
                        nc.vector.tensor_copy(vt[:, sy, nh * 512:(nh + 1) * 512], ps[:])

            # ---- main loop over query windows of 512 rows ----
            for w in range(NW):
                xw = xwp.tile([128, KQ, 512], F32R)
                nc.sync.dma_start(out=xw[:], in_=chunked(xT[:, w * 512:(w + 1) * 512]))

                # qT[d, q] = Wq.T @ xw + bq
                qt = qtp.tile([128, H, 512], F32R)
                for m in range(H):
                    ps = ps_a.tile([128, 512], F32, tag="a")
                    for k in range(KQ):
                        nc.tensor.matmul(ps[:], wq_t[:, k, m * 128:(m + 1) * 128],
                                         xw[:, k, :], start=(k == 0), stop=(k == KQ - 1))
                    nc.scalar.activation(out=qt[:, m, :], in_=ps[:], func=Identity,
                                         bias=bq_t[:, m:m + 1], scale=1.0)

                at = atp.tile([128, H, 512], F32R)
                for h in range(H):
                    # scoresT + exp, one Sy-chunk of 128 at a time
                    ex = exp_p.tile([128, SY // 128, 512], F32R)
                    for kc in range(SY // 128):
                        ps = ps_sc.tile([128, 512], F32, tag="sc")
                        nc.tensor.matmul(ps[:], kt[:, h, kc * 128:(kc + 1) * 128],
                                         qt[:, h, :], start=True, stop=True)
                        nc.scalar.activation(out=ex[:, kc, :], in_=ps[:], func=Exp)
                    # softmax denominator via ones-matmul partition reduction
                    cs_ps = ps_cs.tile([1, 512], F32, tag="cs")
                    for kc in range(SY // 128):
                        nc.tensor.matmul(cs_ps[:], ones_col[:], ex[:, kc, :],
                                         start=(kc == 0), stop=(kc == SY // 128 - 1))
                    cs = csp.tile([1, 512], F32, tag="cs_f32")
                    nc.vector.reciprocal(out=cs[:], in_=cs_ps[:])
                    cs_r = csp.tile([1, 512], F32R, tag="cs_f32r")
                    nc.vector.tensor_copy(cs_r[:], cs[:])
                    bc_ps = ps_bc.tile([128, 512], F32, tag="bc")
                    nc.tensor.matmul(bc_ps[:], ones_row[:], cs_r[:], start=True, stop=True)
                    # aT[d, q] = v_h.T @ eT, then normalize
                    at_ps = ps_at.tile([128, 512], F32, tag="at")
                    for kc in range(SY // 128):
                        nc.tensor.matmul(at_ps[:], vt[:, kc, h * 128:(h + 1) * 128],
                                         ex[:, kc, :], start=(kc == 0),
                                         stop=(kc == SY // 128 - 1))
                    nc.scalar.activation(out=at[:, h, :], in_=at_ps[:], func=Copy)
                    nc.vector.tensor_mul(at[:, h, :], at[:, h, :], bc_ps[:])

                # out[q, n] = sum_h aT_h.T @ Wo_h + bo_eff
                for qc in range(4):
                    for nh in range(2):
                        ps = ps_a.tile([128, 512], F32, tag="a")
                        for h in range(H):
                            nc.tensor.matmul(ps[:], at[:, h, qc * 128:(qc + 1) * 128],
                                             wo_t[:, h, nh * 512:(nh + 1) * 512],
                                             start=(h == 0), stop=(h == H - 1))
                        fo = fop.tile([128, 512], F32)
                        nc.vector.tensor_add(fo[:], ps[:], bo_bc[:, nh * 512:(nh + 1) * 512])
                        r0 = w * 512 + qc * 128
                        nc.sync.dma_start(out=out[r0:r0 + 128, nh * 512:(nh + 1) * 512],
                                          in_=fo[:])
    nc.finalize()
    return nc


def _prep_inputs(x, y, Wq, bq, Wk, bk, Wv, bv, Wo, bo):
    x = np.ascontiguousarray(np.asarray(x, dtype=np.float32))
    y = np.asarray(y, dtype=np.float32).reshape(B, SY, DC)
    Wq = np.ascontiguousarray(np.asarray(Wq, dtype=np.float32))
    Wk = np.asarray(Wk, dtype=np.float32)
    Wv = np.ascontiguousarray(np.asarray(Wv, dtype=np.float32))
    Wo = np.ascontiguousarray(np.asarray(Wo, dtype=np.float32))
    scale = 1.0 / math.sqrt(DH)
    wk_s = np.ascontiguousarray(Wk * scale)
    bk_s = np.asarray(bk, dtype=np.float32) * scale
    bo_eff = (np.asarray(bv, dtype=np.float64) @ np.asarray(Wo, dtype=np.float64)
              + np.asarray(bo, dtype=np.float64)).astype(np.float32)
    ones = np.ones(128, dtype=np.float32)
    bq = np.asarray(bq, dtype=np.float32)

    in_maps = []
    for c in range(NCORES):
        b, hf = divmod(c, NCORES // B)
        xs = x[b, hf * ROWS:(hf + 1) * ROWS, :]
        in_maps.append({
            "xT": np.ascontiguousarray(xs.T),
            "yT": np.ascontiguousarray(y[b].T),
            "wq": Wq, "wk": wk_s, "wv": Wv, "wo": Wo,
            "bq": bq, "bk": bk_s, "bo": bo_eff, "ones": ones,
        })
    return in_maps


def _run(inputs, trace=False):
    _install_shims()
    from concourse.bass_utils import run_bass_kernel_spmd
    if "nc" not in _NC_CACHE:
        _NC_CACHE["nc"] = _build_nc()
    nc = _NC_CACHE["nc"]
    in_maps = _prep_inputs(**inputs)
    res = run_bass_kernel_spmd(nc, in_maps, list(range(NCORES)), trace=trace)
    outf = np.empty((B, SX, DE), dtype=np.float32)
    for c in range(NCORES):
        b, hf = divmod(c, NCORES // B)
        outf[b, hf * ROWS:(hf + 1) * ROWS, :] = res.results[c]["out"]
    return outf, res


def kernel(**inputs):
    out, _ = _run(inputs, trace=False)
    return out


# revision 5
# speedup vs baseline: 1.2189x; 1.2189x over previous
"""Trainium2 Bass kernel for nn_CrossAttention (B=4, Sx=4096, Sy=512, D=1024, H=8).

Sharding: 8 cores = (batch, query-half). Each core handles 2048 query rows of one
batch; K/V projections for that batch are computed locally (replicated across the
2 cores sharing a batch). The output projection is fully local, so no collectives
are needed; each core writes its own [2048, 1024] output slice.

Layouts are arranged so no on-device transposes are needed:
  qT[d, q]   = Wq.T @ xT        (xT pre-transposed on host)
  kT[d, s]   = (Wk/sqrt(dh)).T @ yT
  v[s, d]    = yT.T @ Wv
  scT[s, q]  = kT_h_chunk.T @ qT_h            (per head, Sy chunks of 128)
  eT         = exp(scT)                        (no max-subtract: |scores| ~ O(1))
  Z[h, q]    = onehot_h.T @ eT  (accumulated for all 8 heads into one [8,512] PSUM
               tile -> ONE batched reciprocal per window instead of 32 serial ones)
  aT[d, q]   = v_chunk.T @ eT, then aT *= (1/Z_h) broadcast via K=1 ones-matmul
  out[q, n]  = sum_h aT_h_chunk.T @ Wo_h + (bv @ Wo + bo)
"""
import sys
import types
import math
import numpy as np

sys.path.insert(0, "/opt/trn_rl_repo")

B, SX, SY, DE, DC, H, DH = 4, 4096, 512, 1024, 768, 8, 128
NCORES = 8
ROWS = B * SX // NCORES      # 2048 query rows per core
NW = ROWS // 512             # 4 windows of 512 rows
KQ = DE // 128               # 8 k-chunks for q/out projections
KY = DC // 128               # 6 k-chunks for k/v projections
SC = SY // 128               # 4 Sy chunks

MM_DTYPE = "bf16"            # "bf16" | "f32r"  (matmul operand precision)


def _install_shims():
    """antenv.axon_hooks is missing in this image; register the NTFF profile hook
    so trace=True works, and neuter the fish-bucket artifact upload."""
    if "antenv.axon_hooks" in sys.modules:
        return
    import antenv
    mod = types.ModuleType("antenv.axon_hooks")
    _h = [None]
    mod.set_axon_ntff_profile_hook = lambda h: _h.__setitem__(0, h)
    mod.get_axon_ntff_profile_hook = lambda: _h[0]
    sys.modules["antenv.axon_hooks"] = mod
    antenv.axon_hooks = mod
    try:
        from trn_agent_boot.trn_boot import _ntff_profile_via_ctypes
        mod.set_axon_ntff_profile_hook(
            _ntff_profile_via_ctypes("/opt/axon/libaxon_pjrt.so"))
    except Exception:
        pass
    from concourse import bass_utils
    bass_utils.upload_artifacts = lambda tmpdir: "local://" + tmpdir


_NC_CACHE = {}


def _build_nc(mm_dtype):
    from concourse import bacc, mybir
    from concourse.tile import TileContext

    F32 = mybir.dt.float32
    MMD = mybir.dt.bfloat16 if mm_dtype == "bf16" else mybir.dt.float32r
    Identity = mybir.ActivationFunctionType.Identity
    Copy = mybir.ActivationFunctionType.Copy
    Exp = mybir.ActivationFunctionType.Exp

    nc = bacc.Bacc(None, target_bir_lowering=False)
    xT = nc.dram_tensor("xT", [DE, ROWS], MMD, kind="ExternalInput")
    yT = nc.dram_tensor("yT", [DC, SY], MMD, kind="ExternalInput")
    wq = nc.dram_tensor("wq", [DE, DE], MMD, kind="ExternalInput")
    wk = nc.dram_tensor("wk", [DC, DE], MMD, kind="ExternalInput")
    wv = nc.dram_tensor("wv", [DC, DE], MMD, kind="ExternalInput")
    wo = nc.dram_tensor("wo", [DE, DE], MMD, kind="ExternalInput")
    bq_d = nc.dram_tensor("bq", [DE], F32, kind="ExternalInput")
    bk_d = nc.dram_tensor("bk", [DE], F32, kind="ExternalInput")
    bo_d = nc.dram_tensor("bo", [DE], F32, kind="ExternalInput")
    oh_d = nc.dram_tensor("oh", [128, H * H], MMD, kind="ExternalInput")
    sel_d = nc.dram_tensor("sel", [8, H * 128], MMD, kind="ExternalInput")
    out = nc.dram_tensor("out", [ROWS, DE], F32, kind="ExternalOutput")

    def chunked(ap, p=128):
        # DRAM [K, N] -> [p, K/p, N] view with the 128-partition dim first
        return ap.rearrange("(c p) n -> c p n", p=p).transpose([1, 0, 2])

    with TileContext(nc) as tc:
        with (
            tc.tile_pool(name="consts", bufs=1) as consts,
            tc.tile_pool(name="xwp", bufs=2) as xwp,
            tc.tile_pool(name="qtp", bufs=2) as qtp,
            tc.tile_pool(name="exp_p", bufs=3) as exp_p,
            tc.tile_pool(name="atp", bufs=2) as atp,
            tc.tile_pool(name="fop", bufs=3) as fop,
            tc.tile_pool(name="csp", bufs=2) as csp,
            tc.tile_pool(name="ps_a", bufs=2, space="PSUM") as ps_a,
            tc.tile_pool(name="ps_sc", bufs=2, space="PSUM") as ps_sc,
            tc.tile_pool(name="ps_at", bufs=2, space="PSUM") as ps_at,
            tc.tile_pool(name="ps_z", bufs=1, space="PSUM") as ps_z,
            tc.tile_pool(name="ps_bc", bufs=1, space="PSUM") as ps_bc,
        ):
            # ---- resident constants ----
            wq_t = consts.tile([128, KQ, DE], MMD)
            wo_t = consts.tile([128, KQ, DE], MMD)
            kt = consts.tile([128, H, SY], MMD)       # kT: [d-part, head, Sy]
            vt = consts.tile([128, SC, DE], MMD)      # v: [Sy-part, Sy-chunk, d]
            bo_bc = consts.tile([128, DE], F32)
            bq_t = consts.tile([128, KQ], F32)
            bk_t = consts.tile([128, KQ], F32)
            oh_t = consts.tile([128, H, H], MMD)      # onehot: col h of slice [:,h,:]
            sel_t = consts.tile([8, H, 128], MMD)     # row-selector: row h of [:,h,:]

            nc.sync.dma_start(out=wq_t[:], in_=chunked(wq[:]))
            nc.sync.dma_start(out=wo_t[:], in_=chunked(wo[:]))
            nc.sync.dma_start(out=bo_bc[:], in_=bo_d[:].partition_broadcast(128))
            nc.sync.dma_start(out=bq_t[:], in_=bq_d[:].rearrange("(m p) -> p m", p=128))
            nc.sync.dma_start(out=bk_t[:], in_=bk_d[:].rearrange("(m p) -> p m", p=128))
            nc.sync.dma_start(out=oh_t[:], in_=oh_d[:].rearrange("p (h m) -> p h m", h=H))
            nc.sync.dma_start(out=sel_t[:], in_=sel_d[:].rearrange("p (h m) -> p h m", h=H))

            with tc.tile_pool(name="prologue", bufs=1) as pro:
                yt = pro.tile([128, KY, SY], MMD, tag="yt")
                nc.sync.dma_start(out=yt[:], in_=chunked(yT[:]))
                # stage Wk then Wv in 512-column halves through one shared slot
                for half in range(2):
                    wst = pro.tile([128, KY, 512], MMD, tag="wstage")
                    nc.sync.dma_start(out=wst[:], in_=chunked(wk[:, half * 512:(half + 1) * 512]))
                    # kT[d, s] = (Wk').T @ yT + bk'   (m-tiles covered by this half)
                    for mh in range(4):
                        m = half * 4 + mh
                        ps = ps_sc.tile([128, SY], F32, tag="sc")
                        for k in range(KY):
                            nc.tensor.matmul(ps[:], wst[:, k, mh * 128:(mh + 1) * 128],
                                             yt[:, k, :], start=(k == 0), stop=(k == KY - 1))
                        nc.scalar.activation(out=kt[:, m, :], in_=ps[:], func=Identity,
                                             bias=bk_t[:, m:m + 1], scale=1.0)
                for nh in range(2):
                    wst = pro.tile([128, KY, 512], MMD, tag="wstage")
                    nc.sync.dma_start(out=wst[:], in_=chunked(wv[:, nh * 512:(nh + 1) * 512]))
                    # v[s, d] = yT.T @ Wv  (bv folded into bo_eff on host)
                    for sy in range(SC):
                        ps = ps_at.tile([128, 512], F32, tag="at")
                        for k in range(KY):
                            nc.tensor.matmul(ps[:], yt[:, k, sy * 128:(sy + 1) * 128],
                                             wst[:, k, :], start=(k == 0), stop=(k == KY - 1))
                        nc.vector.tensor_copy(vt[:, sy, nh * 512:(nh + 1) * 512], ps[:])

            # ---- main loop over query windows of 512 rows ----
            for w in range(NW):
                xw = xwp.tile([128, KQ, 512], MMD)
                nc.sync.dma_start(out=xw[:], in_=chunked(xT[:, w * 512:(w + 1) * 512]))

                # qT[d, q] = Wq.T @ xw + bq
                qt = qtp.tile([128, H, 512], MMD)
                for m in range(H):
                    ps = ps_a.tile([128, 512], F32, tag="a")
                    for k in range(KQ):
                        nc.tensor.matmul(ps[:], wq_t[:, k, m * 128:(m + 1) * 128],
                                         xw[:, k, :], start=(k == 0), stop=(k == KQ - 1))
                    nc.scalar.activation(out=qt[:, m, :], in_=ps[:], func=Identity,
                                         bias=bq_t[:, m:m + 1], scale=1.0)

                at = atp.tile([128, H, 512], MMD)
                z_ps = ps_z.tile([8, 512], F32, tag="z")
                for h in range(H):
                    # scoresT + exp, one Sy-chunk of 128 at a time
                    ex = exp_p.tile([128, SC, 512], MMD)
                    for kc in range(SC):
                        ps = ps_sc.tile([128, 512], F32, tag="sc")
                        nc.tensor.matmul(ps[:], kt[:, h, kc * 128:(kc + 1) * 128],
                                         qt[:, h, :], start=True, stop=True)
                        nc.scalar.activation(out=ex[:, kc, :], in_=ps[:], func=Exp)
                    # softmax denominators for all heads -> one shared [8,512] tile
                    for kc in range(SC):
                        nc.tensor.matmul(z_ps[:], oh_t[:, h, :], ex[:, kc, :],
                                         start=(h == 0 and kc == 0),
                                         stop=(h == H - 1 and kc == SC - 1))
                    # aT[d, q] = v_h.T @ eT (normalized after the head loop)
                    at_ps = ps_at.tile([128, 512], F32, tag="at")
                    for kc in range(SC):
                        nc.tensor.matmul(at_ps[:], vt[:, kc, h * 128:(h + 1) * 128],
                                         ex[:, kc, :], start=(kc == 0),
                                         stop=(kc == SC - 1))
                    nc.scalar.activation(out=at[:, h, :], in_=at_ps[:], func=Copy)

                # batched reciprocal of all 8 heads' denominators, then normalize
                zr = csp.tile([8, 512], F32, tag="zr")
                nc.vector.reciprocal(out=zr[:], in_=z_ps[:])
                zr_m = csp.tile([8, 512], MMD, tag="zr_m")
                nc.vector.tensor_copy(zr_m[:], zr[:])
                for h in range(H):
                    bc_ps = ps_bc.tile([128, 512], F32, tag="bc")
                    nc.tensor.matmul(bc_ps[:], sel_t[:, h, :], zr_m[:],
                                     start=True, stop=True)
                    nc.vector.tensor_mul(at[:, h, :], at[:, h, :], bc_ps[:])

                # out[q, n] = sum_h aT_h.T @ Wo_h + bo_eff
                for qc in range(4):
                    for nh in range(2):
                        ps = ps_a.tile([128, 512], F32, tag="a")
                        for h in range(H):
                            nc.tensor.matmul(ps[:], at[:, h, qc * 128:(qc + 1) * 128],
                                             wo_t[:, h, nh * 512:(nh + 1) * 512],
                                             start=(h == 0), stop=(h == H - 1))
                        fo = fop.tile([128, 512], F32)
                        nc.vector.tensor_add(fo[:], ps[:], bo_bc[:, nh * 512:(nh + 1) * 512])
                        r0 = w * 512 + qc * 128
                        nc.sync.dma_start(out=out[r0:r0 + 128, nh * 512:(nh + 1) * 512],
                                          in_=fo[:])
    nc.finalize()
    return nc


def _to_mm(a, mm_dtype):
    if mm_dtype == "bf16":
        import ml_dtypes
        return np.ascontiguousarray(a).astype(ml_dtypes.bfloat16)
    return np.ascontiguousarray(a.astype(np.float32))


def _prep_inputs(mm_dtype, x, y, Wq, bq, Wk, bk, Wv, bv, Wo, bo):
    x = np.asarray(x, dtype=np.float32)
    y = np.asarray(y, dtype=np.float32).reshape(B, SY, DC)
    scale = 1.0 / math.sqrt(DH)
    Wq_m = _to_mm(np.asarray(Wq, np.float32), mm_dtype)
    wk_m = _to_mm(np.asarray(Wk, np.float32) * scale, mm_dtype)
    wv_m = _to_mm(np.asarray(Wv, np.float32), mm_dtype)
    wo_m = _to_mm(np.asarray(Wo, np.float32), mm_dtype)
    bk_s = np.asarray(bk, dtype=np.float32) * scale
    bo_eff = (np.asarray(bv, dtype=np.float64) @ np.asarray(Wo, dtype=np.float64)
              + np.asarray(bo, dtype=np.float64)).astype(np.float32)
    oh = np.zeros((128, H, H), np.float32)
    for h in range(H):
        oh[:, h, h] = 1.0
    oh = _to_mm(oh.reshape(128, H * H), mm_dtype)
    sel = np.zeros((8, H, 128), np.float32)
    for h in range(H):
        sel[h, h, :] = 1.0
    sel = _to_mm(sel.reshape(8, H * 128), mm_dtype)
    bq = np.asarray(bq, dtype=np.float32)

    in_maps = []
    for c in range(NCORES):
        b, hf = divmod(c, NCORES // B)
        xs = x[b, hf * ROWS:(hf + 1) * ROWS, :]
        in_maps.append({
            "xT": _to_mm(xs.T, mm_dtype),
            "yT": _to_mm(y[b].T, mm_dtype),
            "wq": Wq_m, "wk": wk_m, "wv": wv_m, "wo": wo_m,
            "bq": bq, "bk": bk_s, "bo": bo_eff, "oh": oh, "sel": sel,
        })
    return in_maps


def _run(inputs, trace=False, mm_dtype=None):
    _install_shims()
    from concourse.bass_utils import run_bass_kernel_spmd
    mm_dtype = mm_dtype or MM_DTYPE
    if mm_dtype not in _NC_CACHE:
        _NC_CACHE[mm_dtype] = _build_nc(mm_dtype)
    nc = _NC_CACHE[mm_dtype]
    in_maps = _prep_inputs(mm_dtype, **inputs)
    res = run_bass_kernel_spmd(nc, in_maps, list(range(NCORES)), trace=trace)
    outf = np.empty((B, SX, DE), dtype=np.float32)
    for c in range(NCORES):
        b, hf = divmod(c, NCORES // B)
        outf[b, hf * ROWS:(hf + 1) * ROWS, :] = res.results[c]["out"]
    return outf, res


def kernel(**inputs):
    out, _ = _run(inputs, trace=False)
    return out


# revision 6
# speedup vs baseline: 1.4016x; 1.1499x over previous
"""Trainium2 Bass kernel for nn_CrossAttention (B=4, Sx=4096, Sy=512, D=1024, H=8).

Sharding: 8 cores = (batch, query-half). Each core handles 2048 query rows of one
batch; K/V projections for that batch are computed locally (replicated across the
2 cores sharing a batch). The output projection is fully local, so no collectives
are needed; each core writes its own [2048, 1024] output slice.

Layouts are arranged so no on-device transposes are needed:
  qT[d, q]   = Wq.T @ xT        (xT pre-transposed on host)
  kT[d, s]   = (Wk/sqrt(dh)).T @ yT
  v[s, d]    = yT.T @ Wv
  scT[s, q]  = kT_h_chunk.T @ qT_h            (per head, Sy chunks of 128)
  eT         = exp(scT)                        (no max-subtract: |scores| ~ O(1))
  Z[h, q]    = onehot_h.T @ eT  (accumulated for all 8 heads into one [8,512] PSUM
               tile -> ONE batched reciprocal per window instead of 32 serial ones)
  aT[d, q]   = v_chunk.T @ eT, then aT *= (1/Z_h) broadcast via K=1 ones-matmul
  out[q, n]  = sum_h aT_h_chunk.T @ Wo_h + (bv @ Wo + bo)
"""
import sys
import types
import math
import numpy as np

sys.path.insert(0, "/opt/trn_rl_repo")

B, SX, SY, DE, DC, H, DH = 4, 4096, 512, 1024, 768, 8, 128
NCORES = 8
ROWS = B * SX // NCORES      # 2048 query rows per core
NW = ROWS // 512             # 4 windows of 512 rows
KQ = DE // 128               # 8 k-chunks for q/out projections
KY = DC // 128               # 6 k-chunks for k/v projections
SC = SY // 128               # 4 Sy chunks

MM_DTYPE = "bf16"            # "bf16" | "f32r"  (matmul operand precision)


def _install_shims():
    """antenv.axon_hooks is missing in this image; register the NTFF profile hook
    so trace=True works, and neuter the fish-bucket artifact upload."""
    if "antenv.axon_hooks" in sys.modules:
        return
    import antenv
    mod = types.ModuleType("antenv.axon_hooks")
    _h = [None]
    mod.set_axon_ntff_profile_hook = lambda h: _h.__setitem__(0, h)
    mod.get_axon_ntff_profile_hook = lambda: _h[0]
    sys.modules["antenv.axon_hooks"] = mod
    antenv.axon_hooks = mod
    try:
        from trn_agent_boot.trn_boot import _ntff_profile_via_ctypes
        mod.set_axon_ntff_profile_hook(
            _ntff_profile_via_ctypes("/opt/axon/libaxon_pjrt.so"))
    except Exception:
        pass
    from concourse import bass_utils
    bass_utils.upload_artifacts = lambda tmpdir: "local://" + tmpdir


_NC_CACHE = {}


def _build_nc(mm_dtype):
    from concourse import bacc, mybir
    from concourse.tile import TileContext

    F32 = mybir.dt.float32
    MMD = mybir.dt.bfloat16 if mm_dtype == "bf16" else mybir.dt.float32r
    Identity = mybir.ActivationFunctionType.Identity
    Copy = mybir.ActivationFunctionType.Copy
    Exp = mybir.ActivationFunctionType.Exp

    nc = bacc.Bacc(None, target_bir_lowering=False)
    xT = nc.dram_tensor("xT", [DE, ROWS], MMD, kind="ExternalInput")
    yT = nc.dram_tensor("yT", [DC, SY], MMD, kind="ExternalInput")
    wq = nc.dram_tensor("wq", [DE, DE], MMD, kind="ExternalInput")
    wk = nc.dram_tensor("wk", [DC, DE], MMD, kind="ExternalInput")
    wv = nc.dram_tensor("wv", [DC, DE], MMD, kind="ExternalInput")
    wo = nc.dram_tensor("wo", [DE, DE], MMD, kind="ExternalInput")
    bq_d = nc.dram_tensor("bq", [DE], F32, kind="ExternalInput")
    bk_d = nc.dram_tensor("bk", [DE], F32, kind="ExternalInput")
    bo_d = nc.dram_tensor("bo", [DE], F32, kind="ExternalInput")
    oh_d = nc.dram_tensor("oh", [128, H * H], MMD, kind="ExternalInput")
    sel_d = nc.dram_tensor("sel", [8, H * 128], MMD, kind="ExternalInput")
    out = nc.dram_tensor("out", [ROWS, DE], F32, kind="ExternalOutput")

    def chunked(ap, p=128):
        # DRAM [K, N] -> [p, K/p, N] view with the 128-partition dim first
        return ap.rearrange("(c p) n -> c p n", p=p).transpose([1, 0, 2])

    with TileContext(nc) as tc:
        with (
            tc.tile_pool(name="consts", bufs=1) as consts,
            tc.tile_pool(name="xwp", bufs=2) as xwp,
            tc.tile_pool(name="qtp", bufs=2) as qtp,
            tc.tile_pool(name="exp_p", bufs=3) as exp_p,
            tc.tile_pool(name="atp", bufs=2) as atp,
            tc.tile_pool(name="fop", bufs=3) as fop,
            tc.tile_pool(name="csp", bufs=2) as csp,
            tc.tile_pool(name="ps_a", bufs=2, space="PSUM") as ps_a,
            tc.tile_pool(name="ps_sc", bufs=2, space="PSUM") as ps_sc,
            tc.tile_pool(name="ps_at", bufs=2, space="PSUM") as ps_at,
            tc.tile_pool(name="ps_z", bufs=1, space="PSUM") as ps_z,
            tc.tile_pool(name="ps_bc", bufs=1, space="PSUM") as ps_bc,
        ):
            # ---- resident constants ----
            wq_t = consts.tile([128, KQ, DE], MMD)
            wo_t = consts.tile([128, KQ, DE], MMD)
            kt = consts.tile([128, H, SY], MMD)       # kT: [d-part, head, Sy]
            vt = consts.tile([128, SC, DE], MMD)      # v: [Sy-part, Sy-chunk, d]
            bo_bc = consts.tile([128, DE], F32)
            bq_t = consts.tile([128, KQ], F32)
            bk_t = consts.tile([128, KQ], F32)
            oh_t = consts.tile([128, H, H], MMD)      # onehot: col h of slice [:,h,:]
            sel_t = consts.tile([8, H, 128], MMD)     # row-selector: row h of [:,h,:]

            nc.sync.dma_start(out=wq_t[:], in_=chunked(wq[:]))
            nc.sync.dma_start(out=wo_t[:], in_=chunked(wo[:]))
            nc.sync.dma_start(out=bo_bc[:], in_=bo_d[:].partition_broadcast(128))
            nc.sync.dma_start(out=bq_t[:], in_=bq_d[:].rearrange("(m p) -> p m", p=128))
            nc.sync.dma_start(out=bk_t[:], in_=bk_d[:].rearrange("(m p) -> p m", p=128))
            nc.sync.dma_start(out=oh_t[:], in_=oh_d[:].rearrange("p (h m) -> p h m", h=H))
            nc.sync.dma_start(out=sel_t[:], in_=sel_d[:].rearrange("p (h m) -> p h m", h=H))

            with tc.tile_pool(name="prologue", bufs=1) as pro:
                yt = pro.tile([128, KY, SY], MMD, tag="yt")
                nc.sync.dma_start(out=yt[:], in_=chunked(yT[:]))
                # stage Wk then Wv in 512-column halves through one shared slot
                for half in range(2):
                    wst = pro.tile([128, KY, 512], MMD, tag="wstage")
                    nc.sync.dma_start(out=wst[:], in_=chunked(wk[:, half * 512:(half + 1) * 512]))
                    # kT[d, s] = (Wk').T @ yT + bk'   (m-tiles covered by this half)
                    for mh in range(4):
                        m = half * 4 + mh
                        ps = ps_sc.tile([128, SY], F32, tag="sc")
                        for k in range(KY):
                            nc.tensor.matmul(ps[:], wst[:, k, mh * 128:(mh + 1) * 128],
                                             yt[:, k, :], start=(k == 0), stop=(k == KY - 1))
                        nc.scalar.activation(out=kt[:, m, :], in_=ps[:], func=Identity,
                                             bias=bk_t[:, m:m + 1], scale=1.0)
                for nh in range(2):
                    wst = pro.tile([128, KY, 512], MMD, tag="wstage")
                    nc.sync.dma_start(out=wst[:], in_=chunked(wv[:, nh * 512:(nh + 1) * 512]))
                    # v[s, d] = yT.T @ Wv  (bv folded into bo_eff on host)
                    for sy in range(SC):
                        ps = ps_at.tile([128, 512], F32, tag="at")
                        for k in range(KY):
                            nc.tensor.matmul(ps[:], yt[:, k, sy * 128:(sy + 1) * 128],
                                             wst[:, k, :], start=(k == 0), stop=(k == KY - 1))
                        nc.vector.tensor_copy(vt[:, sy, nh * 512:(nh + 1) * 512], ps[:])

            # ---- main loop over query windows of 512 rows ----
            # Window epilogue (normalize + out-proj) is deferred and emitted
            # after the NEXT window's qT matmuls, so the reciprocal/broadcast
            # chain overlaps PE work instead of stalling it.
            pending = None  # (w, at, z_ps) awaiting normalize + out-proj

            def emit_epilogue(p):
                pw, p_at, p_z = p
                zr = csp.tile([8, 512], F32, tag="zr")
                nc.vector.reciprocal(out=zr[:], in_=p_z[:])
                zr_m = csp.tile([8, 512], MMD, tag="zr_m")
                nc.vector.tensor_copy(zr_m[:], zr[:])
                for h in range(H):
                    bc_ps = ps_bc.tile([128, 512], F32, tag="bc")
                    nc.tensor.matmul(bc_ps[:], sel_t[:, h, :], zr_m[:],
                                     start=True, stop=True)
                    nc.vector.tensor_mul(p_at[:, h, :], p_at[:, h, :], bc_ps[:])
                # out[q, n] = sum_h aT_h.T @ Wo_h + bo_eff
                for qc in range(4):
                    for nh in range(2):
                        ps = ps_a.tile([128, 512], F32, tag="a")
                        for h in range(H):
                            nc.tensor.matmul(ps[:], p_at[:, h, qc * 128:(qc + 1) * 128],
                                             wo_t[:, h, nh * 512:(nh + 1) * 512],
                                             start=(h == 0), stop=(h == H - 1))
                        fo = fop.tile([128, 512], F32)
                        nc.vector.tensor_add(fo[:], ps[:], bo_bc[:, nh * 512:(nh + 1) * 512])
                        r0 = pw * 512 + qc * 128
                        nc.sync.dma_start(out=out[r0:r0 + 128, nh * 512:(nh + 1) * 512],
                                          in_=fo[:])

            for w in range(NW):
                xw = xwp.tile([128, KQ, 512], MMD)
                nc.sync.dma_start(out=xw[:], in_=chunked(xT[:, w * 512:(w + 1) * 512]))

                # qT[d, q] = Wq.T @ xw + bq
                qt = qtp.tile([128, H, 512], MMD)
                for m in range(H):
                    ps = ps_a.tile([128, 512], F32, tag="a")
                    for k in range(KQ):
                        nc.tensor.matmul(ps[:], wq_t[:, k, m * 128:(m + 1) * 128],
                                         xw[:, k, :], start=(k == 0), stop=(k == KQ - 1))
                    nc.scalar.activation(out=qt[:, m, :], in_=ps[:], func=Identity,
                                         bias=bq_t[:, m:m + 1], scale=1.0)

                if pending is not None:
                    emit_epilogue(pending)

                at = atp.tile([128, H, 512], MMD)
                z_ps = ps_z.tile([8, 512], F32, tag="z")
                for h in range(H):
                    # scoresT + exp, one Sy-chunk of 128 at a time
                    ex = exp_p.tile([128, SC, 512], MMD)
                    for kc in range(SC):
                        ps = ps_sc.tile([128, 512], F32, tag="sc")
                        nc.tensor.matmul(ps[:], kt[:, h, kc * 128:(kc + 1) * 128],
                                         qt[:, h, :], start=True, stop=True)
                        nc.scalar.activation(out=ex[:, kc, :], in_=ps[:], func=Exp)
                    # softmax denominators for all heads -> one shared [8,512] tile
                    for kc in range(SC):
                        nc.tensor.matmul(z_ps[:], oh_t[:, h, :], ex[:, kc, :],
                                         start=(h == 0 and kc == 0),
                                         stop=(h == H - 1 and kc == SC - 1))
                    # aT[d, q] = v_h.T @ eT (normalized in the deferred epilogue)
                    at_ps = ps_at.tile([128, 512], F32, tag="at")
                    for kc in range(SC):
                        nc.tensor.matmul(at_ps[:], vt[:, kc, h * 128:(h + 1) * 128],
                                         ex[:, kc, :], start=(kc == 0),
                                         stop=(kc == SC - 1))
                    nc.vector.tensor_copy(at[:, h, :], at_ps[:])
                pending = (w, at, z_ps)

            emit_epilogue(pending)
    nc.finalize()
    return nc


def _to_mm(a, mm_dtype):
    if mm_dtype == "bf16":
        import ml_dtypes
        return np.ascontiguousarray(a).astype(ml_dtypes.bfloat16)
    return np.ascontiguousarray(a.astype(np.float32))


def _prep_inputs(mm_dtype, x, y, Wq, bq, Wk, bk, Wv, bv, Wo, bo):
    x = np.asarray(x, dtype=np.float32)
    y = np.asarray(y, dtype=np.float32).reshape(B, SY, DC)
    scale = 1.0 / math.sqrt(DH)
    Wq_m = _to_mm(np.asarray(Wq, np.float32), mm_dtype)
    wk_m = _to_mm(np.asarray(Wk, np.float32) * scale, mm_dtype)
    wv_m = _to_mm(np.asarray(Wv, np.float32), mm_dtype)
    wo_m = _to_mm(np.asarray(Wo, np.float32), mm_dtype)
    bk_s = np.asarray(bk, dtype=np.float32) * scale
    bo_eff = (np.asarray(bv, dtype=np.float64) @ np.asarray(Wo, dtype=np.float64)
              + np.asarray(bo, dtype=np.float64)).astype(np.float32)
    oh = np.zeros((128, H, H), np.float32)
    for h in range(H):
        oh[:, h, h] = 1.0
    oh = _to_mm(oh.reshape(128, H * H), mm_dtype)
    sel = np.zeros((8, H, 128), np.float32)
    for h in range(H):
        sel[h, h, :] = 1.0
    sel = _to_mm(sel.reshape(8, H * 128), mm_dtype)
    bq = np.asarray(bq, dtype=np.float32)

    in_maps = []
    for c in range(NCORES):
        b, hf = divmod(c, NCORES // B)
        xs = x[b, hf * ROWS:(hf + 1) * ROWS, :]
        in_maps.append({
            "xT": _to_mm(xs.T, mm_dtype),
            "yT": _to_mm(y[b].T, mm_dtype),
            "wq": Wq_m, "wk": wk_m, "wv": wv_m, "wo": wo_m,
            "bq": bq, "bk": bk_s, "bo": bo_eff, "oh": oh, "sel": sel,
        })
    return in_maps


def _run(inputs, trace=False, mm_dtype=None):
    _install_shims()
    from concourse.bass_utils import run_bass_kernel_spmd
    mm_dtype = mm_dtype or MM_DTYPE
    if mm_dtype not in _NC_CACHE:
        _NC_CACHE[mm_dtype] = _build_nc(mm_dtype)
    nc = _NC_CACHE[mm_dtype]
    in_maps = _prep_inputs(mm_dtype, **inputs)
    res = run_bass_kernel_spmd(nc, in_maps, list(range(NCORES)), trace=trace)
    outf = np.empty((B, SX, DE), dtype=np.float32)
    for c in range(NCORES):
        b, hf = divmod(c, NCORES // B)
        outf[b, hf * ROWS:(hf + 1) * ROWS, :] = res.results[c]["out"]
    return outf, res


def kernel(**inputs):
    out, _ = _run(inputs, trace=False)
    return out


# revision 7
# speedup vs baseline: 1.4953x; 1.0668x over previous
"""Trainium2 Bass kernel for nn_CrossAttention (B=4, Sx=4096, Sy=512, D=1024, H=8).

Sharding: 8 cores = (batch, query-half). Each core handles 2048 query rows of one
batch; K/V projections for that batch are computed locally (replicated across the
2 cores sharing a batch). The output projection is fully local, so no collectives
are needed; each core writes its own [2048, 1024] output slice.

Layouts are arranged so no on-device transposes are needed:
  qT[d, q]   = Wq.T @ xT        (xT pre-transposed on host)
  kT[d, s]   = (Wk/sqrt(dh)).T @ yT
  v[s, d]    = yT.T @ Wv
  scT[s, q]  = kT_h_chunk.T @ qT_h            (per head, Sy chunks of 128)
  eT         = exp(scT)                        (no max-subtract: |scores| ~ O(1))
  Z[h, q]    = onehot_h.T @ eT  (accumulated for all 8 heads into one [8,512] PSUM
               tile -> ONE batched reciprocal per window instead of 32 serial ones)
  aT[d, q]   = v_chunk.T @ eT, then aT *= (1/Z_h) broadcast via K=1 ones-matmul
  out[q, n]  = sum_h aT_h_chunk.T @ Wo_h + (bv @ Wo + bo)
"""
import sys
import types
import math
import numpy as np

sys.path.insert(0, "/opt/trn_rl_repo")

B, SX, SY, DE, DC, H, DH = 4, 4096, 512, 1024, 768, 8, 128
NCORES = 8
ROWS = B * SX // NCORES      # 2048 query rows per core
NW = ROWS // 512             # 4 windows of 512 rows
KQ = DE // 128               # 8 k-chunks for q/out projections
KY = DC // 128               # 6 k-chunks for k/v projections
SC = SY // 128               # 4 Sy chunks

MM_DTYPE = "bf16"            # "bf16" | "f32r"  (matmul operand precision)


def _install_shims():
    """antenv.axon_hooks is missing in this image; register the NTFF profile hook
    so trace=True works, and neuter the fish-bucket artifact upload."""
    if "antenv.axon_hooks" in sys.modules:
        return
    import antenv
    mod = types.ModuleType("antenv.axon_hooks")
    _h = [None]
    mod.set_axon_ntff_profile_hook = lambda h: _h.__setitem__(0, h)
    mod.get_axon_ntff_profile_hook = lambda: _h[0]
    sys.modules["antenv.axon_hooks"] = mod
    antenv.axon_hooks = mod
    try:
        from trn_agent_boot.trn_boot import _ntff_profile_via_ctypes
        mod.set_axon_ntff_profile_hook(
            _ntff_profile_via_ctypes("/opt/axon/libaxon_pjrt.so"))
    except Exception:
        pass
    from concourse import bass_utils
    bass_utils.upload_artifacts = lambda tmpdir: "local://" + tmpdir


_NC_CACHE = {}


def _build_nc(mm_dtype):
    from concourse import bacc, mybir
    from concourse.tile import TileContext

    F32 = mybir.dt.float32
    MMD = mybir.dt.bfloat16 if mm_dtype == "bf16" else mybir.dt.float32r
    Identity = mybir.ActivationFunctionType.Identity
    Copy = mybir.ActivationFunctionType.Copy
    Exp = mybir.ActivationFunctionType.Exp

    nc = bacc.Bacc(None, target_bir_lowering=False)
    xT = nc.dram_tensor("xT", [DE, ROWS], MMD, kind="ExternalInput")
    yT = nc.dram_tensor("yT", [DC, SY], MMD, kind="ExternalInput")
    wq = nc.dram_tensor("wq", [DE, DE], MMD, kind="ExternalInput")
    wk = nc.dram_tensor("wk", [DC, DE], MMD, kind="ExternalInput")
    wv = nc.dram_tensor("wv", [DC, DE], MMD, kind="ExternalInput")
    wo = nc.dram_tensor("wo", [DE, DE], MMD, kind="ExternalInput")
    bq_d = nc.dram_tensor("bq", [DE], F32, kind="ExternalInput")
    bk_d = nc.dram_tensor("bk", [DE], F32, kind="ExternalInput")
    bo_d = nc.dram_tensor("bo", [DE], F32, kind="ExternalInput")
    oh_d = nc.dram_tensor("oh", [128, H * H], MMD, kind="ExternalInput")
    sel_d = nc.dram_tensor("sel", [8, H * 128], MMD, kind="ExternalInput")
    out = nc.dram_tensor("out", [ROWS, DE], F32, kind="ExternalOutput")

    def chunked(ap, p=128):
        # DRAM [K, N] -> [p, K/p, N] view with the 128-partition dim first
        return ap.rearrange("(c p) n -> c p n", p=p).transpose([1, 0, 2])

    with TileContext(nc) as tc:
        with (
            tc.tile_pool(name="consts", bufs=1) as consts,
            tc.tile_pool(name="xwp", bufs=2) as xwp,
            tc.tile_pool(name="qtp", bufs=2) as qtp,
            tc.tile_pool(name="exp_p", bufs=3) as exp_p,
            tc.tile_pool(name="atp", bufs=2) as atp,
            tc.tile_pool(name="fop", bufs=3) as fop,
            tc.tile_pool(name="csp", bufs=2) as csp,
            tc.tile_pool(name="ps_a", bufs=2, space="PSUM") as ps_a,
            tc.tile_pool(name="ps_sc", bufs=2, space="PSUM") as ps_sc,
            tc.tile_pool(name="ps_at", bufs=2, space="PSUM") as ps_at,
            tc.tile_pool(name="ps_z", bufs=1, space="PSUM") as ps_z,
            tc.tile_pool(name="ps_bc", bufs=1, space="PSUM") as ps_bc,
        ):
            # ---- resident constants ----
            wq_t = consts.tile([128, KQ, DE], MMD)
            wo_t = consts.tile([128, KQ, DE], MMD)
            kt = consts.tile([128, H, SY], MMD)       # kT: [d-part, head, Sy]
            vt = consts.tile([128, SC, DE], MMD)      # v: [Sy-part, Sy-chunk, d]
            bo_bc = consts.tile([128, DE], F32)
            bq_t = consts.tile([128, KQ], F32)
            bk_t = consts.tile([128, KQ], F32)
            oh_t = consts.tile([128, H, H], MMD)      # onehot: col h of slice [:,h,:]
            sel_t = consts.tile([8, H, 128], MMD)     # row-selector: row h of [:,h,:]

            nc.sync.dma_start(out=bq_t[:], in_=bq_d[:].rearrange("(m p) -> p m", p=128))
            nc.sync.dma_start(out=bk_t[:], in_=bk_d[:].rearrange("(m p) -> p m", p=128))
            nc.sync.dma_start(out=oh_t[:], in_=oh_d[:].rearrange("p (h m) -> p h m", h=H))
            nc.sync.dma_start(out=sel_t[:], in_=sel_d[:].rearrange("p (h m) -> p h m", h=H))

            with tc.tile_pool(name="prologue", bufs=2) as pro:
                yt = pro.tile([128, KY, SY], MMD, tag="yt")
                nc.sync.dma_start(out=yt[:], in_=chunked(yT[:]))
                # stage Wk then Wv in 512-column halves through one shared slot
                for half in range(2):
                    wst = pro.tile([128, KY, 512], MMD, tag="wstage")
                    nc.sync.dma_start(out=wst[:], in_=chunked(wk[:, half * 512:(half + 1) * 512]))
                    # kT[d, s] = (Wk').T @ yT + bk'   (m-tiles covered by this half)
                    for mh in range(4):
                        m = half * 4 + mh
                        ps = ps_sc.tile([128, SY], F32, tag="sc")
                        for k in range(KY):
                            nc.tensor.matmul(ps[:], wst[:, k, mh * 128:(mh + 1) * 128],
                                             yt[:, k, :], start=(k == 0), stop=(k == KY - 1))
                        nc.scalar.activation(out=kt[:, m, :], in_=ps[:], func=Identity,
                                             bias=bk_t[:, m:m + 1], scale=1.0)
                for nh in range(2):
                    wst = pro.tile([128, KY, 512], MMD, tag="wstage")
                    nc.sync.dma_start(out=wst[:], in_=chunked(wv[:, nh * 512:(nh + 1) * 512]))
                    # v[s, d] = yT.T @ Wv  (bv folded into bo_eff on host)
                    for sy in range(SC):
                        ps = ps_at.tile([128, 512], F32, tag="at")
                        for k in range(KY):
                            nc.tensor.matmul(ps[:], yt[:, k, sy * 128:(sy + 1) * 128],
                                             wst[:, k, :], start=(k == 0), stop=(k == KY - 1))
                        nc.vector.tensor_copy(vt[:, sy, nh * 512:(nh + 1) * 512], ps[:])

            # Big resident weights are DMA'd AFTER the prologue tiles and the
            # first x window so the serial DMA queue feeds the PE in need-order.
            xw0 = xwp.tile([128, KQ, 512], MMD, tag="xw")
            nc.sync.dma_start(out=xw0[:], in_=chunked(xT[:, 0:512]))
            nc.sync.dma_start(out=wq_t[:], in_=chunked(wq[:]))
            nc.sync.dma_start(out=wo_t[:], in_=chunked(wo[:]))
            nc.sync.dma_start(out=bo_bc[:], in_=bo_d[:].partition_broadcast(128))

            # ---- main loop over query windows of 512 rows ----
            # Window epilogue (normalize + out-proj) is deferred and emitted
            # after the NEXT window's qT matmuls, so the reciprocal/broadcast
            # chain overlaps PE work instead of stalling it.
            pending = None  # (w, at, z_ps) awaiting normalize + out-proj

            def emit_epilogue(p):
                pw, p_at, p_z = p
                zr = csp.tile([8, 512], F32, tag="zr")
                nc.vector.reciprocal(out=zr[:], in_=p_z[:])
                zr_m = csp.tile([8, 512], MMD, tag="zr_m")
                nc.vector.tensor_copy(zr_m[:], zr[:])
                for h in range(H):
                    bc_ps = ps_bc.tile([128, 512], F32, tag="bc")
                    nc.tensor.matmul(bc_ps[:], sel_t[:, h, :], zr_m[:],
                                     start=True, stop=True)
                    nc.vector.tensor_mul(p_at[:, h, :], p_at[:, h, :], bc_ps[:])
                # out[q, n] = sum_h aT_h.T @ Wo_h + bo_eff
                for qc in range(4):
                    for nh in range(2):
                        ps = ps_a.tile([128, 512], F32, tag="a")
                        for h in range(H):
                            nc.tensor.matmul(ps[:], p_at[:, h, qc * 128:(qc + 1) * 128],
                                             wo_t[:, h, nh * 512:(nh + 1) * 512],
                                             start=(h == 0), stop=(h == H - 1))
                        fo = fop.tile([128, 512], F32)
                        nc.vector.tensor_add(fo[:], ps[:], bo_bc[:, nh * 512:(nh + 1) * 512])
                        r0 = pw * 512 + qc * 128
                        nc.sync.dma_start(out=out[r0:r0 + 128, nh * 512:(nh + 1) * 512],
                                          in_=fo[:])

            for w in range(NW):
                if w == 0:
                    xw = xw0
                else:
                    xw = xwp.tile([128, KQ, 512], MMD, tag="xw")
                    nc.sync.dma_start(out=xw[:], in_=chunked(xT[:, w * 512:(w + 1) * 512]))

                # qT[d, q] = Wq.T @ xw + bq
                qt = qtp.tile([128, H, 512], MMD)
                for m in range(H):
                    ps = ps_a.tile([128, 512], F32, tag="a")
                    for k in range(KQ):
                        nc.tensor.matmul(ps[:], wq_t[:, k, m * 128:(m + 1) * 128],
                                         xw[:, k, :], start=(k == 0), stop=(k == KQ - 1))
                    nc.scalar.activation(out=qt[:, m, :], in_=ps[:], func=Identity,
                                         bias=bq_t[:, m:m + 1], scale=1.0)

                if pending is not None:
                    emit_epilogue(pending)

                at = atp.tile([128, H, 512], MMD)
                z_ps = ps_z.tile([8, 512], F32, tag="z")
                for h in range(H):
                    # scoresT + exp, one Sy-chunk of 128 at a time
                    ex = exp_p.tile([128, SC, 512], MMD)
                    for kc in range(SC):
                        ps = ps_sc.tile([128, 512], F32, tag="sc")
                        nc.tensor.matmul(ps[:], kt[:, h, kc * 128:(kc + 1) * 128],
                                         qt[:, h, :], start=True, stop=True)
                        nc.scalar.activation(out=ex[:, kc, :], in_=ps[:], func=Exp)
                    # softmax denominators for all heads -> one shared [8,512] tile
                    for kc in range(SC):
                        nc.tensor.matmul(z_ps[:], oh_t[:, h, :], ex[:, kc, :],
                                         start=(h == 0 and kc == 0),
                                         stop=(h == H - 1 and kc == SC - 1))
                    # aT[d, q] = v_h.T @ eT (normalized in the deferred epilogue)
                    at_ps = ps_at.tile([128, 512], F32, tag="at")
                    for kc in range(SC):
                        nc.tensor.matmul(at_ps[:], vt[:, kc, h * 128:(h + 1) * 128],
                                         ex[:, kc, :], start=(kc == 0),
                                         stop=(kc == SC - 1))
                    nc.vector.tensor_copy(at[:, h, :], at_ps[:])
                pending = (w, at, z_ps)

            emit_epilogue(pending)
    nc.finalize()
    return nc


def _to_mm(a, mm_dtype):
    if mm_dtype == "bf16":
        import ml_dtypes
        return np.ascontiguousarray(a).astype(ml_dtypes.bfloat16)
    return np.ascontiguousarray(a.astype(np.float32))


def _prep_inputs(mm_dtype, x, y, Wq, bq, Wk, bk, Wv, bv, Wo, bo):
    x = np.asarray(x, dtype=np.float32)
    y = np.asarray(y, dtype=np.float32).reshape(B, SY, DC)
    scale = 1.0 / math.sqrt(DH)
    Wq_m = _to_mm(np.asarray(Wq, np.float32), mm_dtype)
    wk_m = _to_mm(np.asarray(Wk, np.float32) * scale, mm_dtype)
    wv_m = _to_mm(np.asarray(Wv, np.float32), mm_dtype)
    wo_m = _to_mm(np.asarray(Wo, np.float32), mm_dtype)
    bk_s = np.asarray(bk, dtype=np.float32) * scale
    bo_eff = (np.asarray(bv, dtype=np.float64) @ np.asarray(Wo, dtype=np.float64)
              + np.asarray(bo, dtype=np.float64)).astype(np.float32)
    oh = np.zeros((128, H, H), np.float32)
    for h in range(H):
        oh[:, h, h] = 1.0
    oh = _to_mm(oh.reshape(128, H * H), mm_dtype)
    sel = np.zeros((8, H, 128), np.float32)
    for h in range(H):
        sel[h, h, :] = 1.0
    sel = _to_mm(sel.reshape(8, H * 128), mm_dtype)
    bq = np.asarray(bq, dtype=np.float32)

    in_maps = []
    for c in range(NCORES):
        b, hf = divmod(c, NCORES // B)
        xs = x[b, hf * ROWS:(hf + 1) * ROWS, :]
        in_maps.append({
            "xT": _to_mm(xs.T, mm_dtype),
            "yT": _to_mm(y[b].T, mm_dtype),
            "wq": Wq_m, "wk": wk_m, "wv": wv_m, "wo": wo_m,
            "bq": bq, "bk": bk_s, "bo": bo_eff, "oh": oh, "sel": sel,
        })
    return in_maps


def _run(inputs, trace=False, mm_dtype=None):
    _install_shims()
    from concourse.bass_utils import run_bass_kernel_spmd
    mm_dtype = mm_dtype or MM_DTYPE
    if mm_dtype not in _NC_CACHE:
        _NC_CACHE[mm_dtype] = _build_nc(mm_dtype)
    nc = _NC_CACHE[mm_dtype]
    in_maps = _prep_inputs(mm_dtype, **inputs)
    res = run_bass_kernel_spmd(nc, in_maps, list(range(NCORES)), trace=trace)
    outf = np.empty((B, SX, DE), dtype=np.float32)
    for c in range(NCORES):
        b, hf = divmod(c, NCORES // B)
        outf[b, hf * ROWS:(hf + 1) * ROWS, :] = res.results[c]["out"]
    return outf, res


def kernel(**inputs):
    out, _ = _run(inputs, trace=False)
    return out


# revision 8
# speedup vs baseline: 1.5025x; 1.0048x over previous
"""Trainium2 Bass kernel for nn_CrossAttention (B=4, Sx=4096, Sy=512, D=1024, H=8).

Sharding: 8 cores = (batch, query-half). Each core handles 2048 query rows of one
batch; K/V projections for that batch are computed locally (replicated across the
2 cores sharing a batch). The output projection is fully local, so no collectives
are needed; each core writes its own [2048, 1024] output slice.

Layouts are arranged so no on-device transposes are needed:
  qT[d, q]   = Wq.T @ xT        (xT pre-transposed on host)
  kT[d, s]   = (Wk/sqrt(dh)).T @ yT
  v[s, d]    = yT.T @ Wv
  scT[s, q]  = kT_h_chunk.T @ qT_h            (per head, Sy chunks of 128)
  eT         = exp(scT)                        (no max-subtract: |scores| ~ O(1))
  Z[h, q]    = onehot_h.T @ eT  (accumulated for all 8 heads into one [8,512] PSUM
               tile -> ONE batched reciprocal per window instead of 32 serial ones)
  aT[d, q]   = v_chunk.T @ eT, then aT *= (1/Z_h) broadcast via K=1 ones-matmul
  out[q, n]  = sum_h aT_h_chunk.T @ Wo_h + (bv @ Wo + bo)
"""
import sys
import types
import math
import numpy as np

sys.path.insert(0, "/opt/trn_rl_repo")

B, SX, SY, DE, DC, H, DH = 4, 4096, 512, 1024, 768, 8, 128
NCORES = 8
ROWS = B * SX // NCORES      # 2048 query rows per core
NW = ROWS // 512             # 4 windows of 512 rows
KQ = DE // 128               # 8 k-chunks for q/out projections
KY = DC // 128               # 6 k-chunks for k/v projections
SC = SY // 128               # 4 Sy chunks

MM_DTYPE = "bf16"            # "bf16" | "f32r"  (matmul operand precision)


def _install_shims():
    """antenv.axon_hooks is missing in this image; register the NTFF profile hook
    so trace=True works, and neuter the fish-bucket artifact upload."""
    if "antenv.axon_hooks" in sys.modules:
        return
    import antenv
    mod = types.ModuleType("antenv.axon_hooks")
    _h = [None]
    mod.set_axon_ntff_profile_hook = lambda h: _h.__setitem__(0, h)
    mod.get_axon_ntff_profile_hook = lambda: _h[0]
    sys.modules["antenv.axon_hooks"] = mod
    antenv.axon_hooks = mod
    try:
        from trn_agent_boot.trn_boot import _ntff_profile_via_ctypes
        mod.set_axon_ntff_profile_hook(
            _ntff_profile_via_ctypes("/opt/axon/libaxon_pjrt.so"))
    except Exception:
        pass
    from concourse import bass_utils
    bass_utils.upload_artifacts = lambda tmpdir: "local://" + tmpdir


_NC_CACHE = {}


def _build_nc(mm_dtype):
    from concourse import bacc, mybir
    from concourse.tile import TileContext

    F32 = mybir.dt.float32
    MMD = mybir.dt.bfloat16 if mm_dtype == "bf16" else mybir.dt.float32r
    Identity = mybir.ActivationFunctionType.Identity
    Copy = mybir.ActivationFunctionType.Copy
    Exp = mybir.ActivationFunctionType.Exp

    nc = bacc.Bacc(None, target_bir_lowering=False)
    xT = nc.dram_tensor("xT", [DE, ROWS], MMD, kind="ExternalInput")
    yT = nc.dram_tensor("yT", [DC, SY], MMD, kind="ExternalInput")
    wq = nc.dram_tensor("wq", [DE, DE], MMD, kind="ExternalInput")
    wk = nc.dram_tensor("wk", [DC, DE], MMD, kind="ExternalInput")
    wv = nc.dram_tensor("wv", [DC, DE], MMD, kind="ExternalInput")
    wo = nc.dram_tensor("wo", [DE, DE], MMD, kind="ExternalInput")
    bq_d = nc.dram_tensor("bq", [DE], F32, kind="ExternalInput")
    bk_d = nc.dram_tensor("bk", [DE], F32, kind="ExternalInput")
    bo_d = nc.dram_tensor("bo", [DE], F32, kind="ExternalInput")
    oh_d = nc.dram_tensor("oh", [128, H * H], MMD, kind="ExternalInput")
    sel_d = nc.dram_tensor("sel", [8, H * 128], MMD, kind="ExternalInput")
    out = nc.dram_tensor("out", [ROWS, DE], F32, kind="ExternalOutput")

    def chunked(ap, p=128):
        # DRAM [K, N] -> [p, K/p, N] view with the 128-partition dim first
        return ap.rearrange("(c p) n -> c p n", p=p).transpose([1, 0, 2])

    with TileContext(nc) as tc:
        with (
            tc.tile_pool(name="consts", bufs=1) as consts,
            tc.tile_pool(name="xwp", bufs=2) as xwp,
            tc.tile_pool(name="qtp", bufs=2) as qtp,
            tc.tile_pool(name="exp_p", bufs=4) as exp_p,
            tc.tile_pool(name="atp", bufs=2) as atp,
            tc.tile_pool(name="fop", bufs=3) as fop,
            tc.tile_pool(name="csp", bufs=2) as csp,
            tc.tile_pool(name="ps_a", bufs=2, space="PSUM") as ps_a,
            tc.tile_pool(name="ps_sc", bufs=2, space="PSUM") as ps_sc,
            tc.tile_pool(name="ps_at", bufs=2, space="PSUM") as ps_at,
            tc.tile_pool(name="ps_z", bufs=1, space="PSUM") as ps_z,
            tc.tile_pool(name="ps_bc", bufs=1, space="PSUM") as ps_bc,
        ):
            # ---- resident constants ----
            wq_t = consts.tile([128, KQ, DE], MMD)
            wo_t = consts.tile([128, KQ, DE], MMD)
            kt = consts.tile([128, H, SY], MMD)       # kT: [d-part, head, Sy]
            vt = consts.tile([128, SC, DE], MMD)      # v: [Sy-part, Sy-chunk, d]
            bo_bc = consts.tile([128, DE], F32)
            bq_t = consts.tile([128, KQ], F32)
            bk_t = consts.tile([128, KQ], F32)
            oh_t = consts.tile([128, H, H], MMD)      # onehot: col h of slice [:,h,:]
            sel_t = consts.tile([8, H, 128], MMD)     # row-selector: row h of [:,h,:]

            with tc.tile_pool(name="prologue", bufs=2) as pro:
                yt = pro.tile([128, KY, SY], MMD, tag="yt")
                nc.sync.dma_start(out=yt[:], in_=chunked(yT[:]))
                nc.sync.dma_start(out=bk_t[:], in_=bk_d[:].rearrange("(m p) -> p m", p=128))
                nc.sync.dma_start(out=bq_t[:], in_=bq_d[:].rearrange("(m p) -> p m", p=128))
                nc.sync.dma_start(out=oh_t[:], in_=oh_d[:].rearrange("p (h m) -> p h m", h=H))
                nc.sync.dma_start(out=sel_t[:], in_=sel_d[:].rearrange("p (h m) -> p h m", h=H))
                # stage Wk then Wv in 512-column halves through one shared slot
                for half in range(2):
                    wst = pro.tile([128, KY, 512], MMD, tag="wstage")
                    nc.sync.dma_start(out=wst[:], in_=chunked(wk[:, half * 512:(half + 1) * 512]))
                    # kT[d, s] = (Wk').T @ yT + bk'   (m-tiles covered by this half)
                    for mh in range(4):
                        m = half * 4 + mh
                        ps = ps_sc.tile([128, SY], F32, tag="sc")
                        for k in range(KY):
                            nc.tensor.matmul(ps[:], wst[:, k, mh * 128:(mh + 1) * 128],
                                             yt[:, k, :], start=(k == 0), stop=(k == KY - 1))
                        nc.scalar.activation(out=kt[:, m, :], in_=ps[:], func=Identity,
                                             bias=bk_t[:, m:m + 1], scale=1.0)
                for nh in range(2):
                    wst = pro.tile([128, KY, 512], MMD, tag="wstage")
                    nc.sync.dma_start(out=wst[:], in_=chunked(wv[:, nh * 512:(nh + 1) * 512]))
                    # v[s, d] = yT.T @ Wv  (bv folded into bo_eff on host)
                    for sy in range(SC):
                        ps = ps_at.tile([128, 512], F32, tag="at")
                        for k in range(KY):
                            nc.tensor.matmul(ps[:], yt[:, k, sy * 128:(sy + 1) * 128],
                                             wst[:, k, :], start=(k == 0), stop=(k == KY - 1))
                        nc.vector.tensor_copy(vt[:, sy, nh * 512:(nh + 1) * 512], ps[:])

            # Big resident weights are DMA'd AFTER the prologue tiles and the
            # first x window so the serial DMA queue feeds the PE in need-order.
            xw0 = xwp.tile([128, KQ, 512], MMD, tag="xw")
            nc.sync.dma_start(out=xw0[:], in_=chunked(xT[:, 0:512]))
            nc.sync.dma_start(out=wq_t[:], in_=chunked(wq[:]))
            nc.sync.dma_start(out=wo_t[:], in_=chunked(wo[:]))
            nc.sync.dma_start(out=bo_bc[:], in_=bo_d[:].partition_broadcast(128))

            # ---- main loop over query windows of 512 rows ----
            # Window epilogue (normalize + out-proj) is deferred and emitted
            # after the NEXT window's qT matmuls, so the reciprocal/broadcast
            # chain overlaps PE work instead of stalling it.
            pending = None  # (w, at, z_ps) awaiting normalize + out-proj

            def emit_epilogue(p):
                pw, p_at, p_z = p
                zr = csp.tile([8, 512], F32, tag="zr")
                nc.vector.reciprocal(out=zr[:], in_=p_z[:])
                zr_m = csp.tile([8, 512], MMD, tag="zr_m")
                nc.vector.tensor_copy(zr_m[:], zr[:])
                for h in range(H):
                    bc_ps = ps_bc.tile([128, 512], F32, tag="bc")
                    nc.tensor.matmul(bc_ps[:], sel_t[:, h, :], zr_m[:],
                                     start=True, stop=True)
                    nc.vector.tensor_mul(p_at[:, h, :], p_at[:, h, :], bc_ps[:])
                # out[q, n] = sum_h aT_h.T @ Wo_h + bo_eff
                for qc in range(4):
                    for nh in range(2):
                        ps = ps_a.tile([128, 512], F32, tag="a")
                        for h in range(H):
                            nc.tensor.matmul(ps[:], p_at[:, h, qc * 128:(qc + 1) * 128],
                                             wo_t[:, h, nh * 512:(nh + 1) * 512],
                                             start=(h == 0), stop=(h == H - 1))
                        fo = fop.tile([128, 512], F32)
                        nc.vector.tensor_add(fo[:], ps[:], bo_bc[:, nh * 512:(nh + 1) * 512])
                        r0 = pw * 512 + qc * 128
                        nc.sync.dma_start(out=out[r0:r0 + 128, nh * 512:(nh + 1) * 512],
                                          in_=fo[:])

            for w in range(NW):
                if w == 0:
                    xw = xw0
                else:
                    xw = xwp.tile([128, KQ, 512], MMD, tag="xw")
                    nc.sync.dma_start(out=xw[:], in_=chunked(xT[:, w * 512:(w + 1) * 512]))

                # qT[d, q] = Wq.T @ xw + bq
                qt = qtp.tile([128, H, 512], MMD)
                for m in range(H):
                    ps = ps_a.tile([128, 512], F32, tag="a")
                    for k in range(KQ):
                        nc.tensor.matmul(ps[:], wq_t[:, k, m * 128:(m + 1) * 128],
                                         xw[:, k, :], start=(k == 0), stop=(k == KQ - 1))
                    nc.scalar.activation(out=qt[:, m, :], in_=ps[:], func=Identity,
                                         bias=bq_t[:, m:m + 1], scale=1.0)

                if pending is not None:
                    emit_epilogue(pending)

                at = atp.tile([128, H, 512], MMD)
                z_ps = ps_z.tile([8, 512], F32, tag="z")
                for h in range(H):
                    # scoresT + exp, one Sy-chunk of 128 at a time
                    ex = exp_p.tile([128, SC, 512], MMD)
                    for kc in range(SC):
                        ps = ps_sc.tile([128, 512], F32, tag="sc")
                        nc.tensor.matmul(ps[:], kt[:, h, kc * 128:(kc + 1) * 128],
                                         qt[:, h, :], start=True, stop=True)
                        nc.scalar.activation(out=ex[:, kc, :], in_=ps[:], func=Exp)
                    # softmax denominators for all heads -> one shared [8,512] tile
                    for kc in range(SC):
                        nc.tensor.matmul(z_ps[:], oh_t[:, h, :], ex[:, kc, :],
                                         start=(h == 0 and kc == 0),
                                         stop=(h == H - 1 and kc == SC - 1))
                    # aT[d, q] = v_h.T @ eT (normalized in the deferred epilogue)
                    at_ps = ps_at.tile([128, 512], F32, tag="at")
                    for kc in range(SC):
                        nc.tensor.matmul(at_ps[:], vt[:, kc, h * 128:(h + 1) * 128],
                                         ex[:, kc, :], start=(kc == 0),
                                         stop=(kc == SC - 1))
                    nc.vector.tensor_copy(at[:, h, :], at_ps[:])
                pending = (w, at, z_ps)

            emit_epilogue(pending)
    nc.finalize()
    return nc


def _to_mm(a, mm_dtype):
    if mm_dtype == "bf16":
        import ml_dtypes
        return np.ascontiguousarray(a).astype(ml_dtypes.bfloat16)
    return np.ascontiguousarray(a.astype(np.float32))


def _prep_inputs(mm_dtype, x, y, Wq, bq, Wk, bk, Wv, bv, Wo, bo):
    x = np.asarray(x, dtype=np.float32)
    y = np.asarray(y, dtype=np.float32).reshape(B, SY, DC)
    scale = 1.0 / math.sqrt(DH)
    Wq_m = _to_mm(np.asarray(Wq, np.float32), mm_dtype)
    wk_m = _to_mm(np.asarray(Wk, np.float32) * scale, mm_dtype)
    wv_m = _to_mm(np.asarray(Wv, np.float32), mm_dtype)
    wo_m = _to_mm(np.asarray(Wo, np.float32), mm_dtype)
    bk_s = np.asarray(bk, dtype=np.float32) * scale
    bo_eff = (np.asarray(bv, dtype=np.float64) @ np.asarray(Wo, dtype=np.float64)
              + np.asarray(bo, dtype=np.float64)).astype(np.float32)
    oh = np.zeros((128, H, H), np.float32)
    for h in range(H):
        oh[:, h, h] = 1.0
    oh = _to_mm(oh.reshape(128, H * H), mm_dtype)
    sel = np.zeros((8, H, 128), np.float32)
    for h in range(H):
        sel[h, h, :] = 1.0
    sel = _to_mm(sel.reshape(8, H * 128), mm_dtype)
    bq = np.asarray(bq, dtype=np.float32)

    in_maps = []
    for c in range(NCORES):
        b, hf = divmod(c, NCORES // B)
        xs = x[b, hf * ROWS:(hf + 1) * ROWS, :]
        in_maps.append({
            "xT": _to_mm(xs.T, mm_dtype),
            "yT": _to_mm(y[b].T, mm_dtype),
            "wq": Wq_m, "wk": wk_m, "wv": wv_m, "wo": wo_m,
            "bq": bq, "bk": bk_s, "bo": bo_eff, "oh": oh, "sel": sel,
        })
    return in_maps


def _run(inputs, trace=False, mm_dtype=None):
    _install_shims()
    from concourse.bass_utils import run_bass_kernel_spmd
    mm_dtype = mm_dtype or MM_DTYPE
    if mm_dtype not in _NC_CACHE:
        _NC_CACHE[mm_dtype] = _build_nc(mm_dtype)
    nc = _NC_CACHE[mm_dtype]
    in_maps = _prep_inputs(mm_dtype, **inputs)
    res = run_bass_kernel_spmd(nc, in_maps, list(range(NCORES)), trace=trace)
    outf = np.empty((B, SX, DE), dtype=np.float32)
    for c in range(NCORES):
        b, hf = divmod(c, NCORES // B)
        outf[b, hf * ROWS:(hf + 1) * ROWS, :] = res.results[c]["out"]
    return outf, res


def kernel(**inputs):
    out, _ = _run(inputs, trace=False)
    return out
